# revision 8
# baseline (speedup 1.0000x reference)
"""Causal self-attention (B=4, S=2048, D=1024, H=16) on 8 Trainium2 cores, v2.

Sharding: core c -> (batch b = c//2, head-group g = c%2 of 8 heads).
Same dataflow skeleton as v1 (q^T/k^T projections, transposed scores,
ones-augmented AV for softmax denominators) with these changes:

  - softmax normalization: reciprocal rows r2[1,2,512] are broadcast
    across partitions by two SBUF->SBUF DMAs (stride-0 re-read of the
    partition-0 row) instead of a PE replicate-matmul into a shared psum
    bank; unnormalized y also leaves psum immediately via a DVE copy
    (yu_sb) so the next pair's AV chain never waits on the broadcast
  - output projection double-buffers across two psum banks (four for the
    final batch, when psav is free), is split per contraction tile so
    each matmul waits only its own pair's normalize, and the final batch
    runs m-major so the last broadcast is covered by earlier matmuls
  - diagonal score blocks are trimmed to live columns in the sc matmul,
    the exp, AND the av matmul (the masked prefix is never read), and
    the causal mask is a single [128,128] triangle multiply on the
    boundary subtile
  - qk bias-copies run on DVE (tensor_scalar add) and v copies on ACT,
    so each engine's critical chain stays short
  - the v projection accumulates in the proj psum banks and interleaves
    into early attention as PE filler; qk groups for the next query tile
    and the next pair's tt=0 groups are prefetched as filler too
  - loads are batched 3D DMAs split across the SP and ACT hwdge queues,
    ordered so the v-phase gate lands in a few microseconds

Semaphores: dq SP-DMA completions (+16: loads then stores), da ACT-DMA
completions (+16: wqk/wp loads), db broadcast-chain DMAs (+16),
pc PE (+1), ca ACT (+1), dv DVE (+1), pl Pool (+1).
"""

import os
import sys

sys.path.insert(0, "/opt/trn_rl_repo")

import ml_dtypes
import numpy as np

B, S, D, H = 4, 2048, 1024, 16
HD = D // H          # 64
HPC = H // 2         # 8 heads per core
PD = 512             # local proj contraction (8 heads * 64)
P = 128
NTQ = S // 512       # 4 query tiles of 512
NKT = S // P         # 16 key tiles of 128
NPAIR = 4
DK = D // P          # 8 contraction tiles
N_ESLOT = 6

# SP loads: wv d<4, x-q0 d<4, x-q1 d<4, wqk j<4, masks, bqk, x-h1 d<4
N_SP_LOADS = 7
DQ_V0 = 16 * 2        # wva + xq0a -> vgrp vt<4
DQ_MISC = 16 * 4      # + masks + bqk (exp bias, triangle mask)
DQ_V1 = 16 * 5        # + xq1a -> vgrp vt in [4,8), qk tt<2
DQ_QK = 16 * 6        # + wqka
DQ_XH1 = 16 * 7      # + xh1a -> vt>=8, tt>=2
# ACT loads: wv d>=4, x-q0 d>=4, x-q1 d>=4, wqk j>=4, x-half1 d>=4, wp
DA_V0 = 16 * 2
DA_V1 = 16 * 3
DA_QK = 16 * 4
DA_XH1 = 16 * 5
DA_ALL = 16 * 6

_CACHED = {}


def _plan(n_iter=1):
    """Build per-engine programs with symbolic waits, then resolve to
    semaphore thresholds. n_iter > 1 repeats the compute (not the loads)
    with a coarse all-engine barrier between iterations (timing only)."""
    sp, pe, act, dve, pool = [], [], [], [], []
    pe_unit, act_idx, dve_idx, pool_idx = {}, {}, {}, {}
    n_ca = [0]  # ca increments only on non-load ACT ops
    n_units = [0]
    n_exp = [0]
    n_class = [0]
    n_store = [0]
    n_bc = [0]

    def pop(key, **kw):
        pe_unit[key] = n_units[0]
        n_units[0] += 1
        pe.append({"key": key, "waits": kw.pop("waits", []), **kw})

    def aop(key, **kw):
        if kw.get("kind") != "load":
            n_ca[0] += 1
            act_idx[key] = n_ca[0]
        act.append({"key": key, "waits": kw.pop("waits", []), **kw})

    def dop(key, **kw):
        dve_idx[key] = len(dve)
        dve.append({"key": key, "waits": kw.pop("waits", []), **kw})

    def plop(key, **kw):
        pool_idx[key] = len(pool)
        pool.append({"key": key, "waits": kw.pop("waits", []), **kw})

    # batched 3D-AP loads split across the two hwdge queues (SP, ACT),
    # ordered so the v-phase gate (wv + first x quarter) lands first and
    # the big wqk transfer does not block it on the shared DMA pool
    # tiny loads (masks, bqk) issue first so their fast completions are
    # consistent with every later dq threshold
    for name in ["wva", "xq0a", "masks", "bqk", "xq1a", "wqka", "xh1a"]:
        sp.append({"key": ("load", name), "kind": "load", "name": name,
                   "waits": []})
    for name in ["wvb", "xq0b", "xq1b", "wqkb", "xh1b", "wp"]:
        aop(("load", name), kind="load", name=name)

    def class_slot():
        g = n_class[0]
        n_class[0] += 1
        return g % 4

    def walk_one():
        e_slot_consumer = [None] * N_ESLOT
        slot_consumer = [None, None, None, None]   # psc halves (banks)
        av_bank_consumer = [None, None]            # psav banks
        # pspj banks are shared by interleaved vgrps (drained by vcopy/dv)
        # and pjgrps (drained by oc/pl); entries: ("v", vt) | ("oc", ti, nt)
        pj_bank_tenant = [None, None]

        dop(("vmemset",), kind="vmemset", waits=[])

        def pj_bank_wait(bank):
            t = pj_bank_tenant[bank]
            if t is None:
                return []
            return [("ca" if t[0] == "vcopy" else "dv", t)]

        def emit_v(vt, defer=None):
            bank = vt % 2
            if vt < 4:
                w = [("dq#", DQ_V0), ("da#", DA_V0)]
            elif vt < 8:
                w = [("dq#", DQ_V1), ("da#", DA_V1)]
            else:
                w = [("dq#", DQ_XH1), ("da#", DA_XH1)]
            w += pj_bank_wait(bank)
            pop(("v", vt), kind="vgrp", vt=vt, bank=bank, waits=w)
            pj_bank_tenant[bank] = ("vcopy", vt)

            def copy():
                aop(("vcopy", vt), kind="vcopy", vt=vt, bank=bank,
                    waits=[("pc", ("v", vt))])
            if defer is None:
                copy()
            else:
                defer.append(copy)

        # first four v tiles must precede tq=0 attention; the rest
        # interleave into tq0/tq1 pairs as PE filler
        for vt in range(4):
            emit_v(vt)

        def emit_qk(jt, tt):
            slot = class_slot()
            w = [("da#", DA_QK if tt < 2 else DA_XH1),
                 ("dq#", DQ_QK if tt < 2 else DQ_XH1)]
            if slot_consumer[slot]:
                c = slot_consumer[slot]
                w.append(("dv" if c[0] == "qkc" else "ca", c))
            pop(("qk", jt, tt), kind="qkgrp", jt=jt, tt=tt, slot=slot, waits=w)
            slot_consumer[slot] = ("qkc", jt, tt)
            dop(("qkc", jt, tt), kind="qkcopy", jt=jt, tt=tt, slot=slot,
                waits=[("pc", ("qk", jt, tt)), ("dq#", DQ_MISC)])

        def emit_proj(ti, nt, wide=False):
            tq = ti // 4
            g = n_store[0]
            bank = g % 4 if wide else g % 2
            for m in range(4):
                w = [("dv", ("ym", tq, m, 1))]
                if m == 0:
                    w.append(("da#", DA_ALL))
                    w += pj_bank_wait(bank)
                pop(("pj", ti, nt, m), kind="pjm", ti=ti, nt=nt, m=m,
                    bank=bank, waits=w)

            pj_bank_tenant[bank] = ("oc", ti, nt)
            wo = [("pc", ("pj", ti, nt, 3))]
            if g >= 2:
                # osb slot reuse: that slot's store DMA must have completed
                wo.append(("dq#", 16 * (N_SP_LOADS + g - 1)))
            dop(("oc", ti, nt), kind="oc", ti=ti, nt=nt, bank=bank,
                slot=g % 2, waits=wo)
            ws = [("dv", ("oc", ti, nt))]
            if g >= 2:
                ws.append(("dq#", 16 * (N_SP_LOADS + g - 1)))
            sp.append({"key": ("os", ti, nt), "kind": "ostore", "ti": ti,
                       "nt": nt, "slot": g % 2, "waits": ws})
            n_store[0] += 1

        # ---- attention, software-pipelined across head pairs
        pairs = [(tq, p) for tq in range(NTQ) for p in range(NPAIR)]
        pair_state = {}

        def stage_a(i):
            # allocations + qk (tq0) + exp/ms emissions + first two sc
            tq, p = pairs[i]
            n_kt = 4 * (tq + 1)
            if (tq, p) == (0, 0):
                emit_qk(0, 0)
                emit_qk(NPAIR, 0)
            sc_slot, sc_tenant, exp_slot = {}, {}, {}

            def alloc_kt(kt):
                ps = class_slot() // 2 * 2
                class_slot()
                sc_slot[kt] = ps
                sc_tenant[kt] = (slot_consumer[ps], slot_consumer[ps + 1])
                # kt0's exp is split per half; record per-half tenants so a
                # later sc claiming only h0's bank waits just the h0 half
                if kt == 0:
                    slot_consumer[ps] = ("exph0", tq, p)
                else:
                    slot_consumer[ps] = ("exp", tq, p, kt)
                slot_consumer[ps + 1] = ("exp", tq, p, kt)

            # kt0/kt1 claim slots first; the pair's qk(tq+1) groups (emitted
            # right after sc0/sc1 as pair-start PE filler) claim next, so
            # their tenants are early exps; then the remaining kts
            alloc_kt(0)
            if n_kt > 1:
                alloc_kt(1)
            def emit_exp(kt):
                s = n_exp[0] % N_ESLOT
                n_exp[0] += 1
                exp_slot[kt] = s
                r = kt - 4 * tq
                if kt == 0:
                    # split the pair's first exp into per-half ops so the
                    # sc(kt2) psc-slot handoff unblocks half an exp earlier
                    wh = [("pc", ("sc", tq, p, 0, 0))]
                    if e_slot_consumer[s]:
                        wh.append(("pc", e_slot_consumer[s]))
                    aop(("exph0", tq, p), kind="exph", tq=tq, p=p, kt=0,
                        r=0, hh=0, slot=sc_slot[0] // 2, eslot=s, waits=wh)
                    w = [("pc", ("sc", tq, p, 0, 1))]
                    if e_slot_consumer[s]:
                        w.append(("pc", e_slot_consumer[s]))
                    aop(("exp", tq, p, 0), kind="exph", tq=tq, p=p, kt=0,
                        r=0, hh=1, slot=sc_slot[0] // 2, eslot=s, waits=w)
                else:
                    w = [("pc", ("sc", tq, p, kt, 1))]
                    if e_slot_consumer[s]:
                        w.append(("pc", e_slot_consumer[s]))
                    aop(("exp", tq, p, kt), kind="exp", tq=tq, p=p,
                        kt=kt, r=max(r, 0), slot=sc_slot[kt] // 2, eslot=s,
                        waits=w)
                e_slot_consumer[s] = ("av", tq, p, kt, 1)

            def emit_exp2(kt):
                # merged exp over (kt, kt+1): one ACT op covering all four
                # psc banks and two adjacent e slots
                s = n_exp[0] % N_ESLOT
                n_exp[0] += 2
                exp_slot[kt] = s
                exp_slot[kt + 1] = s + 1
                r = max(kt - 4 * tq, 0)
                w = [("pc", ("sc", tq, p, kt + 1, 1))]
                for ss in (s, s + 1):
                    if e_slot_consumer[ss]:
                        w.append(("pc", e_slot_consumer[ss]))
                aop(("exp", tq, p, kt), kind="exp2", tq=tq, p=p, kt=kt,
                    r=r, eslot=s, waits=w)
                act_idx[("exp", tq, p, kt + 1)] = act_idx[("exp", tq, p, kt)]
                e_slot_consumer[s] = ("av", tq, p, kt, 1)
                e_slot_consumer[s + 1] = ("av", tq, p, kt + 1, 1)

            emit_exp(0)
            if n_kt > 1:
                emit_exp(1)
            pair_state[i] = (sc_slot, sc_tenant, exp_slot, alloc_kt,
                             emit_exp, emit_exp2)

        def emit_sc(i, kt):
            tq, p = pairs[i]
            sc_slot, sc_tenant = pair_state[i][0], pair_state[i][1]
            r = kt - 4 * tq
            for h in (0, 1):
                half = sc_slot[kt] + h
                w = [("dv", ("qkc", p, tq)),
                     ("dv", ("qkc", NPAIR + p, kt // 4))]
                if sc_tenant[kt][h]:
                    c = sc_tenant[kt][h]
                    w.append(("dv" if c[0] == "qkc" else "ca", c))
                pop(("sc", tq, p, kt, h), kind="sc", tq=tq, p=p,
                    kt=kt, h=h, r=max(r, 0), slot=half, waits=w)

        def emit_av(i, kt):
            tq, p = pairs[i]
            exp_slot = pair_state[i][2]
            n_kt = 4 * (tq + 1)
            r = kt - 4 * tq
            if r >= 0:
                # triangle mask emitted here (not in stage_a) so pair i+1's
                # tris cannot block pair i's recip/ycopy in DVE order.
                dop(("tri", tq, p, kt), kind="tri", r=r,
                    eslot=exp_slot[kt],
                    waits=[("ca", ("exp", tq, p, kt)), ("dq#", DQ_MISC)])
            for h in (0, 1):
                w = [("ca", ("exp", tq, p, kt))]
                if r >= 0:
                    w.append(("dv", ("tri", tq, p, kt)))
                w.append(("ca", ("vcopy", kt)))
                if kt == 0 and av_bank_consumer[h]:
                    w.append(("dv", av_bank_consumer[h]))
                pop(("av", tq, p, kt, h), kind="av", tq=tq, p=p, kt=kt,
                    h=h, bank=h, r=max(r, 0), eslot=exp_slot[kt],
                    start=(kt == 0), stop=(kt == n_kt - 1), waits=w)

        for i, (tq, p) in enumerate(pairs):
            n_kt = 4 * (tq + 1)
            deferred = []
            stage_a(i)
            emit_sc(i, 0)
            if n_kt > 1:
                emit_sc(i, 1)
            _, _, _, alloc_kt, emit_exp, emit_exp2 = pair_state[i]
            for kt in range(2, n_kt):
                alloc_kt(kt)
            for kt in range(2, n_kt):
                emit_exp(kt)
            for kt in range(n_kt):
                if kt + 2 < n_kt:
                    emit_sc(i, kt + 2)
                emit_av(i, kt)
                # mid-pair PE filler at tq0: a v tile + the NEXT
                # pair's tt=0 qk groups (so its first sc isn't gated on
                # a just-finished qkcopy)
                if tq == 0 and kt == 1:
                    # ACT-side copy deferred to pair end so it does not sit
                    # between this pair's exps and delay the diag chain
                    emit_v(4 + p, defer=deferred)
                    if p + 1 < NPAIR:
                        emit_qk(p + 1, 0)
                        emit_qk(NPAIR + p + 1, 0)

            av_bank_consumer[0] = ("yc", tq, p, 0)
            av_bank_consumer[1] = ("yc", tq, p, 1)
            for copy in deferred:
                copy()

            for h in (0, 1):
                dop(("recip", tq, p, h), kind="recip", h=h, bank=h,
                    waits=[("pc", ("av", tq, p, n_kt - 1, h))])
            # unnormalized y leaves psum immediately so the next pair's
            # first av is gated on a short DVE copy, not the bcast roundtrip
            for h in (0, 1):
                dop(("yc", tq, p, h), kind="yc", h=h, bank=h,
                    waits=[("pc", ("av", tq, p, n_kt - 1, h))])

            # reciprocal rows broadcast across partitions via two
            # SBUF->SBUF DMAs on the SP queue (stride-0 re-read of the
            # partition-0 row; no DRAM bounce)
            g = n_bc[0]
            n_bc[0] += 1
            for h in (0, 1):
                sp.append({"key": ("bc", tq, p, h), "kind": "bc", "h": h,
                           "waits": [("dv", ("recip", tq, p, h))]})

            if tq == 0:
                emit_v(8 + 2 * p)
                emit_v(9 + 2 * p)
            if tq + 1 < NTQ:
                emit_qk(p, tq + 1)
                emit_qk(NPAIR + p, tq + 1)

            # lagged proj emitted BEFORE ym so its oc drain is not queued
            # behind ym's wait on the broadcast roundtrip in DVE order
            if tq >= 1:
                for k in (0, 1):
                    gi = p * 2 + k
                    emit_proj((tq - 1) * 4 + gi // 2, gi % 2)

            if i == len(pairs) - 1:
                # final batch: psav is free, rotate over four banks; the
                # pjm matmuls are emitted before the ym ops (m=3 waits ym
                # by semaphore), but the oc/store drains must follow ym in
                # DVE queue order
                pj_bank_tenant.extend([av_bank_consumer[0],
                                       av_bank_consumer[1]])
                final = []
                for gi in range(8):
                    ti2, nt2 = tq * 4 + gi // 2, gi % 2
                    g2 = n_store[0]
                    final.append((ti2, nt2, g2, g2 % 4))
                    n_store[0] += 1
                # m-major waves of four groups: all m=0 matmuls, then m=1,
                # ... so the m=3 wave (gated on the last ym) has ~2.5us of
                # earlier matmuls covering the broadcast roundtrip
                for wave in (final[0:4], final[4:8]):
                    for m in range(4):
                        for ti2, nt2, g2, bank2 in wave:
                            w2 = [("dv", ("ym", tq, m, 1))]
                            if m == 0:
                                w2 += pj_bank_wait(bank2)
                            pop(("pj", ti2, nt2, m), kind="pjm", ti=ti2,
                                nt=nt2, m=m, bank=bank2, waits=w2)
                    for ti2, nt2, g2, bank2 in wave:
                        pj_bank_tenant[bank2] = ("oc", ti2, nt2)

            for h in (0, 1):
                dop(("ym", tq, p, h), kind="ym", tq=tq, p=p, h=h, bank=h,
                    waits=[("db#", 32 * g + 16 * (h + 1))])

            if i == len(pairs) - 1:
                for ti2, nt2, g2, bank2 in final:
                    wo = [("pc", ("pj", ti2, nt2, 3))]
                    if g2 >= 2:
                        wo.append(("dq#", 16 * (N_SP_LOADS + g2 - 1)))
                    dop(("oc", ti2, nt2), kind="oc", ti=ti2, nt=nt2,
                        bank=bank2, slot=g2 % 2, waits=wo)
                    ws = [("dv", ("oc", ti2, nt2))]
                    if g2 >= 2:
                        ws.append(("dq#", 16 * (N_SP_LOADS + g2 - 1)))
                    sp.append({"key": ("os", ti2, nt2), "kind": "ostore",
                               "ti": ti2, "nt": nt2, "slot": g2 % 2,
                               "waits": ws})

    resolved_upto = {}

    def resolve(name, prog):
        for op in prog[resolved_upto.get(name, 0):]:
            out = []
            for sem, ref in op["waits"]:
                if sem.endswith("#"):
                    out.append((sem[:-1], ref))
                elif sem == "pc":
                    out.append(("pc", pe_unit[ref] + 1))
                elif sem == "ca":
                    out.append(("ca", act_idx[ref]))
                elif sem == "dv":
                    out.append(("dv", dve_idx[ref] + 1))
                elif sem == "pl":
                    out.append(("pl", pool_idx[ref] + 1))
                else:
                    raise AssertionError(sem)
            merged = {}
            for s, v in out:
                merged[s] = max(merged.get(s, 0), v)
            op["waits"] = [(s, v) for s, v in merged.items() if v > 0]
        resolved_upto[name] = len(prog)

    for i in range(n_iter):
        barrier = None
        if i > 0:
            barrier = {
                "pe": [("ca#", n_ca[0]), ("dv#", len(dve))],
                "act": [("pc#", n_units[0]), ("dv#", len(dve))],
                "dve": [("pc#", n_units[0]), ("ca#", n_ca[0]),
                        ("dq#", 16 * (N_SP_LOADS + n_store[0]))],
                "pool": [("pc#", n_units[0]), ("ca#", n_ca[0]),
                         ("dv#", len(dve))],
            }
        pre = {"pe": len(pe), "act": len(act), "dve": len(dve),
               "pool": len(pool)}
        walk_one()
        if barrier:
            for name, prog in (("pe", pe), ("act", act), ("dve", dve),
                               ("pool", pool)):
                if len(prog) > pre[name]:
                    prog[pre[name]]["waits"] = (
                        list(barrier[name]) + prog[pre[name]]["waits"])
        for name, prog in (("sp", sp), ("pe", pe), ("act", act),
                           ("dve", dve), ("pool", pool)):
            resolve(name, prog)

    for prog in (sp, pe, act, dve, pool):
        last = {}
        for op in prog:
            kept = []
            for s, v in op["waits"]:
                if v > last.get(s, -1):
                    kept.append((s, v))
                    last[s] = v
            op["waits"] = kept

    return {"sp": sp, "pe": pe, "act": act, "dve": dve, "pool": pool}


def _build_program(n_iter=1):
    import concourse.bass as bass
    import concourse.mybir as mybir
    from contextlib import ExitStack

    f32 = mybir.dt.float32
    bf16 = mybir.dt.bfloat16
    AF = mybir.ActivationFunctionType
    MUL = mybir.AluOpType.mult
    ADD = mybir.AluOpType.add

    plan = _plan(n_iter)
    nc = bass.Bass()

    xT = nc.dram_tensor("xT", [D, S], bf16, kind="ExternalInput")
    wqk = nc.dram_tensor("wqk", [D, 2 * PD], bf16, kind="ExternalInput")
    wv = nc.dram_tensor("wv", [D, PD], bf16, kind="ExternalInput")
    wp = nc.dram_tensor("wp", [PD, D], bf16, kind="ExternalInput")
    bqk = nc.dram_tensor("bqk", [P, 9], f32, kind="ExternalInput")
    masks = nc.dram_tensor("masks", [P, P], bf16, kind="ExternalInput")
    out = nc.dram_tensor("out", [S, D], f32, kind="ExternalOutput")

    xT_r = xT.rearrange("(o p) t -> p o t", p=P)
    wqk_r = wqk.rearrange("(o p) j -> p o j", p=P)
    wv_r = wv.rearrange("(o p) j -> p o j", p=P)
    wp_r = wp.rearrange("(o p) n -> p o n", p=P)

    with ExitStack() as ctx:
        x_sb = ctx.enter_context(nc.sbuf_tensor([P, DK, S], bf16))
        wqk_sb = ctx.enter_context(nc.sbuf_tensor([P, DK, 2 * PD], bf16))
        wv_sb = ctx.enter_context(nc.sbuf_tensor([P, DK, PD], bf16))
        wp_sb = ctx.enter_context(nc.sbuf_tensor([P, PD // P, D], bf16))
        b_sb = ctx.enter_context(nc.sbuf_tensor([P, 9], f32))
        m_sb = ctx.enter_context(nc.sbuf_tensor([P, P], bf16))
        qkT_sb = ctx.enter_context(nc.sbuf_tensor([P, 2 * NPAIR, S], bf16))
        v_sb = ctx.enter_context(nc.sbuf_tensor([P, NKT, HPC, 65], bf16))
        y_sb = ctx.enter_context(nc.sbuf_tensor([P, NPAIR, S], bf16))
        e_sb = ctx.enter_context(nc.sbuf_tensor([P, N_ESLOT, 2, 512], bf16))
        r2_sb = ctx.enter_context(nc.sbuf_tensor([1, 2, 512], f32))
        osb = ctx.enter_context(nc.sbuf_tensor([P, 2, 512], f32))
        yu_sb = ctx.enter_context(nc.sbuf_tensor([P, 512], f32))
        bc_sb = ctx.enter_context(nc.sbuf_tensor([P, 512], f32))
        psc = ctx.enter_context(nc.psum_tensor("psc", [P, 4, 512], f32))
        psav = ctx.enter_context(nc.psum_tensor("psav", [P, 2, 512], f32))
        pspj = ctx.enter_context(nc.psum_tensor("pspj", [P, 2, 512], f32))

        def psc_half(s):
            return psc[:, s, :]

        def psc_pair(pair):
            return psc[:, 2 * pair:2 * pair + 2, :]

        dq = ctx.enter_context(nc.semaphore("dq"))
        da = ctx.enter_context(nc.semaphore("da"))
        db = ctx.enter_context(nc.semaphore("db"))
        pc = ctx.enter_context(nc.semaphore("pc"))
        ca = ctx.enter_context(nc.semaphore("ca"))
        dv = ctx.enter_context(nc.semaphore("dv"))
        pl = ctx.enter_context(nc.semaphore("pl"))
        sems = {"dq": dq, "da": da, "db": db, "pc": pc, "ca": ca,
                "dv": dv, "pl": pl}
        block = ctx.enter_context(nc.Block())

        sp_load_map = {
            "masks": (m_sb[:], masks[:]),
            "bqk": (b_sb[:], bqk[:]),
            "wva": (wv_sb[:, 0:4, :], wv_r[:, 0:4, :]),
            "xq0a": (x_sb[:, 0:4, 0:512], xT_r[:, 0:4, 0:512]),
            "xq1a": (x_sb[:, 0:4, 512:1024], xT_r[:, 0:4, 512:1024]),
            "wqka": (wqk_sb[:, :, 0:512], wqk_r[:, :, 0:512]),
            "xh1a": (x_sb[:, 0:4, 1024:2048], xT_r[:, 0:4, 1024:2048]),
        }
        act_load_map = {
            "wvb": (wv_sb[:, 4:8, :], wv_r[:, 4:8, :]),
            "xq0b": (x_sb[:, 4:8, 0:512], xT_r[:, 4:8, 0:512]),
            "xq1b": (x_sb[:, 4:8, 512:1024], xT_r[:, 4:8, 512:1024]),
            "wqkb": (wqk_sb[:, :, 512:1024], wqk_r[:, :, 512:1024]),
            "xh1b": (x_sb[:, 4:8, 1024:2048], xT_r[:, 4:8, 1024:2048]),
            "wp": (wp_sb[:], wp_r[:]),
        }

        def do_waits(eng, op):
            for s, v in op["waits"]:
                eng.wait_ge(sems[s], v)

        @block.sync
        def _(eng):
            for op in plan["sp"]:
                do_waits(eng, op)
                k = op["kind"]
                if k == "load":
                    dst, src = sp_load_map[op["name"]]
                    eng.dma_start(dst, src).then_inc(dq, 16)
                elif k == "bc":
                    h = op["h"]
                    bsrc = r2_sb[0:1, h, None, :].to_broadcast((1, 64, 512))
                    eng.dma_start(bc_sb[64 * h:64 * h + 64, :], bsrc
                                  ).then_inc(db, 16)
                else:  # ostore
                    ti, nt, sl = op["ti"], op["nt"], op["slot"]
                    eng.dma_start(
                        out[ti * P:(ti + 1) * P, nt * 512:(nt + 1) * 512],
                        osb[:, sl, :],
                    ).then_inc(dq, 16)

        def mm_split(out_ap, lhsT, rhs, **kw):
            nc.tensor.ldweights(lhsT, tile_position=kw.get("tile_position"))
            mm = nc.tensor.matmul(out_ap, lhsT, rhs, **kw)
            mm.ins.ldweights = False
            return mm

        @block.tensor
        def _(eng):
            for op in plan["pe"]:
                do_waits(eng, op)
                k = op["kind"]
                if k == "vgrp":
                    vt, bank = op["vt"], op["bank"]
                    for d in range(DK):
                        mm = mm_split(
                            pspj[:, bank, :],
                            x_sb[:, d, vt * P:(vt + 1) * P],
                            wv_sb[:, d, :],
                            start=(d == 0), stop=(d == DK - 1),
                        )
                    mm.then_inc(pc, 1)
                elif k == "qkgrp":
                    jt, tt, sl = op["jt"], op["tt"], op["slot"]
                    for d in range(DK):
                        mm = mm_split(
                            psc_half(sl),
                            wqk_sb[:, d, jt * P:(jt + 1) * P],
                            x_sb[:, d, tt * 512:(tt + 1) * 512],
                            start=(d == 0), stop=(d == DK - 1),
                        )
                    mm.then_inc(pc, 1)
                elif k == "sc":
                    tq, p, kt, h, sl, r = (op["tq"], op["p"], op["kt"],
                                           op["h"], op["slot"], op["r"])
                    pr = slice(64 * h, 64 * h + 64)
                    c0 = r * 128
                    mm_split(
                        psc_half(sl)[:, c0:512],
                        qkT_sb[pr, NPAIR + p, kt * P:(kt + 1) * P],
                        qkT_sb[pr, p, tq * 512 + c0:(tq + 1) * 512],
                        start=True, stop=True, tile_position=(64 * h, 0),
                    ).then_inc(pc, 1)
                elif k == "av":
                    kt, h, bank = op["kt"], op["h"], op["bank"]
                    p = op["p"]
                    c0 = op["r"] * 128
                    mm_split(
                        psav[0:65, bank, c0:512],
                        v_sb[:, kt, 2 * p + h, :],
                        e_sb[:, op["eslot"], h, c0:512],
                        start=op["start"], stop=op["stop"],
                    ).then_inc(pc, 1)
                else:  # pjm: one contraction tile of a proj group
                    ti, nt, m, bank = op["ti"], op["nt"], op["m"], op["bank"]
                    dst = (pspj[:, bank, :] if bank < 2
                           else psav[0:128, bank - 2, :])
                    mm_split(
                        dst,
                        y_sb[:, m, ti * P:(ti + 1) * P],
                        wp_sb[:, m, nt * 512:(nt + 1) * 512],
                        start=(m == 0), stop=(m == PD // P - 1),
                    ).then_inc(pc, 1)

        @block.scalar
        def _(eng):
            for op in plan["act"]:
                do_waits(eng, op)
                if op["kind"] == "load":
                    dst, src = act_load_map[op["name"]]
                    eng.dma_start(dst, src).then_inc(da, 16)
                elif op["kind"] == "vcopy":
                    vt, bank = op["vt"], op["bank"]
                    nc.scalar.activation(
                        v_sb[:, vt, :, 0:64],
                        pspj[:, bank, :].rearrange("p (h d) -> p h d", h=HPC),
                        AF.Identity, bias=b_sb[:, 8:9], scale=1.0,
                    ).then_inc(ca, 1)
                elif op["kind"] == "exph":
                    hh = op["hh"]
                    nc.scalar.activation(
                        e_sb[:, op["eslot"], hh, :],
                        psc_pair(op["slot"])[:, hh, :],
                        AF.Exp, bias=b_sb[:, 8:9], scale=0.125,
                    ).then_inc(ca, 1)
                elif op["kind"] == "exp2":
                    c0 = op["r"] * 128
                    nc.scalar.activation(
                        e_sb[:, op["eslot"]:op["eslot"] + 2, :, c0:512],
                        psc[:, :, c0:512],
                        AF.Exp, bias=b_sb[:, 8:9], scale=0.125,
                    ).then_inc(ca, 1)
                else:  # exp over the pair slot, trimmed to live columns
                    r = op["r"]
                    c0 = r * 128
                    nc.scalar.activation(
                        e_sb[:, op["eslot"], :, c0:512],
                        psc_pair(op["slot"])[:, :, c0:512],
                        AF.Exp, bias=b_sb[:, 8:9], scale=0.125,
                    ).then_inc(ca, 1)

        @block.vector
        def _(eng):
            for op in plan["dve"]:
                do_waits(eng, op)
                k = op["kind"]
                if k == "vmemset":
                    nc.vector.memset(v_sb[:, :, :, 64], 1.0).then_inc(dv, 1)
                elif k == "qkcopy":
                    jt, tt, sl = op["jt"], op["tt"], op["slot"]
                    nc.vector.tensor_scalar(
                        qkT_sb[:, jt, tt * 512:(tt + 1) * 512],
                        psc_half(sl), b_sb[:, jt:jt + 1], None, ADD,
                    ).then_inc(dv, 1)
                elif k == "tri":
                    r = op["r"]
                    e = e_sb[:, op["eslot"], :, r * 128:(r + 1) * 128]
                    mb = m_sb[:, None, :].to_broadcast((P, 2, 128))
                    nc.vector.tensor_tensor(e, e, mb, MUL).then_inc(dv, 1)
                elif k == "oc":
                    b = op["bank"]
                    srcb = pspj[:, b, :] if b < 2 else psav[0:128, b - 2, :]
                    nc.vector.tensor_copy(
                        osb[:, op["slot"], :], srcb
                    ).then_inc(dv, 1)
                elif k == "recip":
                    h = op["h"]
                    nc.vector.reciprocal(
                        r2_sb[0:1, h, :], psav[64:65, op["bank"], :]
                    ).then_inc(dv, 1)
                elif k == "yc":
                    rows = slice(64 * op["h"], 64 * op["h"] + 64)
                    nc.vector.tensor_copy(
                        yu_sb[rows, :], psav[0:64, op["bank"], :]
                    ).then_inc(dv, 1)
                else:  # ym
                    h = op["h"]
                    rows = slice(64 * h, 64 * h + 64)
                    nc.vector.tensor_tensor(
                        y_sb[rows, op["p"], op["tq"] * 512:(op["tq"] + 1) * 512],
                        yu_sb[rows, :], bc_sb[rows, :], MUL,
                    ).then_inc(dv, 1)

        if plan["pool"]:
            @block.gpsimd
            def _(eng):
                for op in plan["pool"]:
                    do_waits(eng, op)
                    raise AssertionError("no pool ops expected")

    return nc


def _get_nc(n_iter=1):
    key = f"nc{n_iter}"
    if key not in _CACHED:
        _CACHED[key] = _build_program(n_iter)
    return _CACHED[key]


def _masks_np():
    bf = ml_dtypes.bfloat16
    j = np.arange(P)[:, None]
    i = np.arange(P)[None, :]
    return np.ascontiguousarray((j <= i).astype(bf))


def kernel(x, w_attn, b_attn, w_proj, b_proj):
    from concourse import bass_utils

    bf = ml_dtypes.bfloat16
    nc = _get_nc()
    masks_np = _masks_np()

    x = np.asarray(x)
    w_attn = np.asarray(w_attn)
    b_attn = np.asarray(b_attn, dtype=np.float32)
    w_proj = np.asarray(w_proj)
    b_proj = np.asarray(b_proj, dtype=np.float32)

    in_maps = []
    corrections = []
    for c in range(8):
        b, g = c // 2, c % 2
        heads = np.arange(g * HPC, (g + 1) * HPC)
        cols = (heads[:, None] * HD + np.arange(HD)[None, :]).reshape(-1)
        qk_cols = np.concatenate([cols, D + cols])
        bqk_np = np.zeros((P, 9), np.float32)
        bqk_np[:, 0:8] = b_attn[qk_cols].reshape(8, P).T
        bv = b_attn[2 * D + cols]
        corrections.append(bv @ w_proj[cols, :])
        in_maps.append({
            "xT": np.ascontiguousarray(x[b].T).astype(bf),
            "wqk": np.ascontiguousarray(w_attn[:, qk_cols]).astype(bf),
            "wv": np.ascontiguousarray(w_attn[:, 2 * D + cols]).astype(bf),
            "wp": np.ascontiguousarray(w_proj[cols, :]).astype(bf),
            "bqk": bqk_np,
            "masks": masks_np,
        })

    trace = bool(int(os.environ.get("KERNEL_TRACE", "0")))
    try:
        res = bass_utils.run_bass_kernel_spmd(
            nc, in_maps, core_ids=list(range(8)), trace=trace,
        )
    except Exception:
        import time as _time
        _time.sleep(5)
        res = bass_utils.run_bass_kernel_spmd(
            nc, in_maps, core_ids=list(range(8)), trace=trace,
        )
    _CACHED["last_results"] = res
    _CACHED["last_in_maps"] = in_maps

    outs = [np.asarray(r["out"], dtype=np.float32) for r in res.results]
    full = np.stack([
        outs[2 * b] + outs[2 * b + 1]
        + corrections[2 * b][None, :] + corrections[2 * b + 1][None, :]
        for b in range(B)
    ])
    full += b_proj[None, None, :]
    return full.astype(np.float32)
